# revision 3
# baseline (speedup 1.0000x reference)
"""Trainium2 Bass kernel for nn_CosineLoss: mean_i(1 - output[i, targets[i]]).

v3: block-gather via dma_gather (4 SWDGE instructions instead of 8+).

  - Core c owns rows [c*1024, (c+1)*1024) of `output` ([1024, 32000] f32).
  - The shard is viewed as [128000, 256] f32 blocks (32000 = 125*256, so a
    row is exactly 125 blocks). Instruction g of 4 covers rows
    [256g, 256(g+1)): for item j (row i = 256g + j), the int16 block index
    j*125 + t_i//256 <= 31999 fits the dma_gather int16 index; the in_ap is
    the [g*32000, (g+1)*32000) block slice, so each instruction gathers 256
    1KB blocks (one SWDGE instruction = 994ns fixed + 0.34ns/desc, vs 8
    instructions for indirect_dma_start whose HW cap is 128 descriptors).
  - Item j lands at blocks[j%128, 2g + j//128, :], i.e. row i's block sits
    at blocks[i%128, i//128, :], with x[i, t_i] at offset t_i%256.
  - Select+reduce: host ships a [128, 8, 256] f32 one-hot (1 at t_i%256);
    scalar_tensor_tensor(mult) with accum_out fuses mask-multiply and the
    free-dim sum in one DVE pass per gather chunk, staggered so only the
    last chunk's pass sits behind the final gather.
  - ones-matmul collapses partitions to a [1,1] PSUM scalar -> DMA out.
  - Host sums the 8 partials and returns 1 - total/8192 as a () f32 array.
"""

import numpy as np

from concourse import bacc, bass, mybir
import concourse.tile as tile
from concourse.bass_utils import run_bass_kernel_spmd

N = 8192
C = 32000
NCORES = 8
NL = N // NCORES  # 1024 rows per core
P = 128
F = NL // P  # 8 block slots per partition
B = 256  # block elements (1KB, dma_gather minimum granularity)
G = 4  # dma_gather instructions per core
RG = NL // G  # 256 rows per instruction
BPR = C // B  # 125 blocks per row

_NC_CACHE = {}


def _build():
    nc = bacc.Bacc("TRN2")
    x = nc.dram_tensor("x", [NL * C // B, B], mybir.dt.float32, kind="ExternalInput")
    idx = nc.dram_tensor("idx", [P, G * (RG // 16)], mybir.dt.int16, kind="ExternalInput")
    oh = nc.dram_tensor("oh", [P, F * B], mybir.dt.float32, kind="ExternalInput")
    partial = nc.dram_tensor("partial", [1, 1], mybir.dt.float32, kind="ExternalOutput")

    with tile.TileContext(nc) as tc:
        with (
            tc.tile_pool(name="sbuf", bufs=1) as sbuf,
            tc.tile_pool(name="psum", bufs=1, space="PSUM") as psum,
        ):
            idx_t = sbuf.tile([P, G * (RG // 16)], mybir.dt.int16)
            nc.sync.dma_start(out=idx_t[:], in_=idx[:])
            oh_t = sbuf.tile([P, F, B], mybir.dt.float32)
            nc.sync.dma_start(out=oh_t[:], in_=oh[:])

            ones = sbuf.tile([P, 1], mybir.dt.float32)
            nc.vector.memset(ones[:], 1.0)

            blocks = sbuf.tile([P, F, B], mybir.dt.float32)
            scratch = sbuf.tile([P, F, B], mybir.dt.float32)
            sums = sbuf.tile([P, G], mybir.dt.float32)

            # explicit DMA-completion semaphore: each dma_gather's descriptors
            # bump it by 16 when the data lands; the DVE selects gate on it.
            dsem = nc.alloc_semaphore("gather_dma_sem")
            S = RG // 16  # idx slots per instruction per partition
            for g in range(G):
                nc.gpsimd.dma_gather(
                    blocks[:, 2 * g : 2 * g + 2, :],
                    x[g * RG * BPR : (g + 1) * RG * BPR, :],
                    idx_t[:, g * S : (g + 1) * S],
                    RG,
                    RG,
                    B,
                ).then_inc(dsem, 16)

            for g in range(G):
                nc.vector.wait_ge(dsem, 16 * (g + 1))
                nc.vector.scalar_tensor_tensor(
                    out=scratch[:, 2 * g : 2 * g + 2, :],
                    in0=blocks[:, 2 * g : 2 * g + 2, :],
                    scalar=1.0,
                    in1=oh_t[:, 2 * g : 2 * g + 2, :],
                    op0=mybir.AluOpType.mult,
                    op1=mybir.AluOpType.mult,
                    accum_out=sums[:, g : g + 1],
                )

            red = sbuf.tile([P, 1], mybir.dt.float32)
            nc.vector.tensor_reduce(
                out=red[:],
                in_=sums[:],
                axis=mybir.AxisListType.X,
                op=mybir.AluOpType.add,
            )
            acc = psum.tile([1, 1], mybir.dt.float32)
            nc.tensor.matmul(out=acc[:], lhsT=ones[:], rhs=red[:], start=True, stop=True)
            res = sbuf.tile([1, 1], mybir.dt.float32)
            nc.vector.tensor_copy(out=res[:], in_=acc[:])
            nc.sync.dma_start(out=partial[:], in_=res[:])

    nc.compile()
    return nc


def _get_nc():
    if "nc" not in _NC_CACHE:
        _NC_CACHE["nc"] = _build()
    return _NC_CACHE["nc"]


def _shard(output, targets):
    xs = np.ascontiguousarray(
        output.reshape(NCORES, NL * C // B, B).astype(np.float32, copy=False)
    )
    t = targets.reshape(NCORES, NL).astype(np.int32)

    S = RG // 16
    idx = np.empty((NCORES, P, G * S), dtype=np.int16)
    oh = np.zeros((NCORES, P, F * B), dtype=np.float32)
    j = np.arange(RG, dtype=np.int32)
    i_all = np.arange(NL, dtype=np.int32)
    for c in range(NCORES):
        for g in range(G):
            tg = t[c, g * RG : (g + 1) * RG]
            v = (j * BPR + tg // B).astype(np.int16)  # [RG], item j
            # item j lives at idx[p, g*S + j//16] for p % 16 == j % 16
            wrapped = v.reshape(S, 16).T  # [16, S]
            idx[c, :, g * S : (g + 1) * S] = np.tile(wrapped, (P // 16, 1))
        # one-hot: row i's block is at [i%128, (i//128)*B + t%B]
        oh[c, i_all % P, (i_all // P) * B + t[c] % B] = 1.0
    return xs, idx, oh


def _run(output, targets, **kwargs):
    xs, idx, oh = _shard(output, targets)
    in_maps = [{"x": xs[c], "idx": idx[c], "oh": oh[c]} for c in range(NCORES)]
    return run_bass_kernel_spmd(
        _get_nc(), in_maps, core_ids=list(range(NCORES)), **kwargs
    )


def kernel(output, targets):
    res = _run(output, targets)
    total = sum(float(r["partial"][0, 0]) for r in res.results)
    return np.array(np.float32(1.0) - np.float32(total / N), dtype=np.float32)


# revision 6
# speedup vs baseline: 1.4967x; 1.4967x over previous
"""Trainium2 Bass kernel for nn_CosineLoss: mean_i(1 - output[i, targets[i]]).

Strategy (data-parallel over the batch dim, 8 cores):
  - Core c owns rows [c*1024, (c+1)*1024) of `output` ([1024, 32000] f32 shard)
    plus flat element offsets idx[i] = i*32000 + targets[i] for its rows
    (int32, laid out [128, 8] in SBUF; descriptor address math in the SWDGE
    is integer, while on-device ALU adds go through an fp32 path that
    corrupts indices above 2^24).
  - On device: 8 indirect DMAs (128 descriptors each -- the HW unrolls one
    descriptor per dest partition row, so 128 scattered elements per
    instruction is the cap) gather the 1024 needed f32 elements from HBM
    (4 KB instead of 131 MB).
  - Split free-dim reduce: cols 0:4 reduce while gathers 5-8 are still in
    flight; only the cols 4:8 reduce sits behind the last gather. A single
    [128,2] matmul against ones then collapses partitions, and a final DVE
    reduce of the [1,2] PSUM row gives the partial-sum scalar per core.
  - Host sums the 8 partials and returns 1 - total/8192 as a () f32 array.
"""

import numpy as np

from concourse import bacc, bass, mybir
import concourse.tile as tile
from concourse.bass_utils import run_bass_kernel_spmd

N = 8192
C = 32000
NCORES = 8
NL = N // NCORES  # 1024 rows per core
P = 128
F = NL // P  # 8 gathered elements per partition

_NC_CACHE = {}


def _build():
    # Bacc (not Bass): its compile() runs generate_event_semaphores, which
    # splits multi-sem waits -- walrus codegen allows 1 sync wait per inst.
    nc = bacc.Bacc("TRN2")
    x = nc.dram_tensor("x", [NL, C], mybir.dt.float32, kind="ExternalInput")
    idx = nc.dram_tensor("idx", [P, F], mybir.dt.int32, kind="ExternalInput")
    partial = nc.dram_tensor("partial", [1, 1], mybir.dt.float32, kind="ExternalOutput")

    # Warm the SWDGE ucode before the idx DMA lands: the first indirect DMA
    # pays a ~1us Q7 IRAM load, so issue a tiny dummy gather (offset 0, 2
    # partitions) as a raw instruction BEFORE the Tile body. It only depends
    # on a same-engine memset (program order on gpsimd), so its IRAM load +
    # descriptor gen overlap the idx DMA + its completion-sem propagation
    # instead of delaying the first real gather.
    warm_off = nc.alloc_sbuf_tensor("warm_off", [2, 1], mybir.dt.int32)
    nc.gpsimd.memset(warm_off.ap(), 0)
    warm_out = nc.alloc_sbuf_tensor("warm_out", [2, 1], mybir.dt.float32)
    warm_sem = nc.alloc_semaphore("warm_sem")
    nc.gpsimd.indirect_dma_start(
        out=warm_out.ap(),
        out_offset=None,
        in_=x[:],
        in_offset=bass.IndirectOffsetOnAxis(ap=warm_off.ap(), axis=1),
    ).then_inc(warm_sem, 16)

    with tile.TileContext(nc) as tc:
        with (
            tc.tile_pool(name="sbuf", bufs=1) as sbuf,
            tc.tile_pool(name="psum", bufs=1, space="PSUM") as psum,
        ):
            idx_t = sbuf.tile([P, F], mybir.dt.int32)
            nc.sync.dma_start(out=idx_t[:], in_=idx[:])

            # ones for the partition-reduce matmul: DVE memset overlaps the
            # gathers (DVE is otherwise idle until the first reduce).
            ones = sbuf.tile([P, 1], mybir.dt.float32)
            nc.vector.memset(ones[:], 1.0)

            gathered = sbuf.tile([P, F], mybir.dt.float32)
            for j in range(F):
                nc.gpsimd.indirect_dma_start(
                    out=gathered[:, j : j + 1],
                    out_offset=None,
                    in_=x[:],
                    in_offset=bass.IndirectOffsetOnAxis(
                        ap=idx_t[:, j : j + 1], axis=1
                    ),
                )

            # red[:,0] covers gathers 1-4 and runs while gathers 5-8 are
            # still generating descriptors; only red[:,1] waits on the tail.
            red = sbuf.tile([P, 2], mybir.dt.float32)
            nc.vector.tensor_reduce(
                out=red[:, 0:1],
                in_=gathered[:, 0 : F // 2],
                axis=mybir.AxisListType.X,
                op=mybir.AluOpType.add,
            )
            nc.vector.tensor_reduce(
                out=red[:, 1:2],
                in_=gathered[:, F // 2 : F],
                axis=mybir.AxisListType.X,
                op=mybir.AluOpType.add,
            )

            # partition-reduce via matmul with ones as lhsT: LDWEIGHTS(ones)
            # prefetches early; only the rhs stream (red) sits on the critical
            # path after the reduces. [1,2] PSUM keeps the out DMA at one
            # descriptor after the final DVE collapse.
            acc = psum.tile([1, 2], mybir.dt.float32)
            nc.tensor.matmul(out=acc[:], lhsT=ones[:], rhs=red[:], start=True, stop=True)
            res = sbuf.tile([1, 1], mybir.dt.float32)
            nc.vector.tensor_reduce(
                out=res[:],
                in_=acc[:],
                axis=mybir.AxisListType.X,
                op=mybir.AluOpType.add,
            )
            nc.sync.dma_start(out=partial[:], in_=res[:])

    # run Bacc passes (reg alloc, event-sem splitting); run_bass_via_pjrt
    # serializes the module without calling finalize() on prebuilt modules.
    nc.compile()
    return nc


def _get_nc():
    if "nc" not in _NC_CACHE:
        _NC_CACHE["nc"] = _build()
    return _NC_CACHE["nc"]


def _shard(output, targets):
    xs = np.ascontiguousarray(
        output.reshape(NCORES, NL, C).astype(np.float32, copy=False)
    )
    flat = np.arange(NL, dtype=np.int32) * C + targets.reshape(NCORES, NL).astype(
        np.int32
    )
    return xs, np.ascontiguousarray(flat.reshape(NCORES, P, F))


def _run(output, targets, **kwargs):
    xs, idx = _shard(output, targets)
    in_maps = [{"x": xs[c], "idx": idx[c]} for c in range(NCORES)]
    return run_bass_kernel_spmd(
        _get_nc(), in_maps, core_ids=list(range(NCORES)), **kwargs
    )


def kernel(output, targets):
    res = _run(output, targets)
    total = sum(float(r["partial"][0, 0]) for r in res.results)
    return np.array(np.float32(1.0) - np.float32(total / N), dtype=np.float32)


# revision 8
# speedup vs baseline: 1.6241x; 1.0851x over previous
"""Trainium2 Bass kernel for nn_CosineLoss: mean_i(1 - output[i, targets[i]]).

Strategy (data-parallel over the batch dim, 8 cores):
  - Core c owns rows [c*1024, (c+1)*1024) of `output` ([1024, 32000] f32 shard)
    plus flat element offsets idx[i] = i*32000 + targets[i] for its rows
    (int32, laid out [128, 8] in SBUF; descriptor address math in the SWDGE
    is integer, while on-device ALU adds go through an fp32 path that
    corrupts indices above 2^24).
  - On device: 8 indirect DMAs (128 descriptors each -- the HW unrolls one
    descriptor per dest partition row, so 128 scattered elements per
    instruction is the cap) gather the 1024 needed f32 elements from HBM
    (4 KB instead of 131 MB).
  - Raw bass (no TileContext): a hand-rolled semaphore graph avoids the
    tile-pool exit cleanup (drain + RANGE_CLEAR + barrier sandwich, ~2us of
    the measured window).
  - Split free-dim reduce: cols 0:4 reduce while gathers 5-8 are still in
    flight; only the cols 4:8 reduce sits behind the last gather. A single
    [128,2] matmul against ones then collapses partitions, and a final DVE
    reduce of the [1,2] PSUM row gives the partial-sum scalar per core.
  - Host sums the 8 partials and returns 1 - total/8192 as a () f32 array.
"""

import numpy as np

from concourse import bacc, bass, mybir
from concourse.bass_utils import run_bass_kernel_spmd

N = 8192
C = 32000
NCORES = 8
NL = N // NCORES  # 1024 rows per core
P = 128
F = NL // P  # 8 gathered elements per partition

_NC_CACHE = {}


def _build():
    # Bacc (not Bass): its compile() runs generate_event_semaphores, which
    # splits multi-sem waits -- walrus codegen allows 1 sync wait per inst.
    nc = bacc.Bacc("TRN2")
    x = nc.dram_tensor("x", [NL, C], mybir.dt.float32, kind="ExternalInput")
    idx = nc.dram_tensor("idx", [P, F], mybir.dt.int32, kind="ExternalInput")
    partial = nc.dram_tensor("partial", [1, 1], mybir.dt.float32, kind="ExternalOutput")

    idx_t = nc.alloc_sbuf_tensor("idx_t", [P, F], mybir.dt.int32)
    gath = nc.alloc_sbuf_tensor("gath", [P, F], mybir.dt.float32)
    ones_t = nc.alloc_sbuf_tensor("ones_t", [P, 1], mybir.dt.float32)
    red = nc.alloc_sbuf_tensor("red", [P, 2], mybir.dt.float32)
    res = nc.alloc_sbuf_tensor("res", [1, 1], mybir.dt.float32)
    acc = nc.alloc_psum_tensor("acc", [1, 2], mybir.dt.float32)

    s_idx = nc.alloc_semaphore("s_idx")  # idx DMA completion (+16)
    s_ga = nc.alloc_semaphore("s_ga")  # gathers 1-4 DMA completions (+16 each)
    s_gb = nc.alloc_semaphore("s_gb")  # gathers 5-8 DMA completions (+16 each)
    s_one = nc.alloc_semaphore("s_one")  # ones memset done
    s_v = nc.alloc_semaphore("s_v")  # DVE progress
    s_mm = nc.alloc_semaphore("s_mm")  # matmul (HIGH pass) done

    nc.sync.dma_start(out=idx_t.ap(), in_=idx[:]).then_inc(s_idx, 16)
    nc.vector.memset(ones_t.ap(), 1.0).then_inc(s_one, 1)

    nc.gpsimd.wait_ge(s_idx, 16)
    for j in range(F):
        nc.gpsimd.indirect_dma_start(
            out=gath.ap()[:, j : j + 1],
            out_offset=None,
            in_=x[:],
            in_offset=bass.IndirectOffsetOnAxis(ap=idx_t.ap()[:, j : j + 1], axis=1),
        ).then_inc(s_ga if j < F // 2 else s_gb, 16)

    # red[:,0] covers gathers 1-4 and runs while gathers 5-8 are still
    # generating descriptors; only red[:,1] waits on the tail.
    nc.vector.wait_ge(s_ga, 16 * (F // 2))
    nc.vector.tensor_reduce(
        out=red.ap()[:, 0:1],
        in_=gath.ap()[:, 0 : F // 2],
        axis=mybir.AxisListType.X,
        op=mybir.AluOpType.add,
    ).then_inc(s_v, 1)
    nc.vector.wait_ge(s_gb, 16 * (F - F // 2))
    nc.vector.tensor_reduce(
        out=red.ap()[:, 1:2],
        in_=gath.ap()[:, F // 2 : F],
        axis=mybir.AxisListType.X,
        op=mybir.AluOpType.add,
    ).then_inc(s_v, 1)

    # partition-reduce via matmul with ones as lhsT: LDWEIGHTS(ones)
    # prefetches early (move_matmul_waits_to_ldweights hoists the waits);
    # only the rhs stream (red) sits on the critical path after the reduces.
    nc.tensor.wait_ge(s_one, 1)
    nc.tensor.wait_ge(s_v, 2)
    nc.tensor.matmul(
        out=acc.ap(), lhsT=ones_t.ap(), rhs=red.ap(), start=True, stop=True
    ).then_inc(s_mm, 1)

    nc.vector.wait_ge(s_mm, 1)
    nc.vector.tensor_reduce(
        out=res.ap(),
        in_=acc.ap(),
        axis=mybir.AxisListType.X,
        op=mybir.AluOpType.add,
    ).then_inc(s_v, 1)
    s_out = nc.alloc_semaphore("s_out")  # walrus requires an update on every DMA
    nc.sync.wait_ge(s_v, 3)
    nc.sync.dma_start(out=partial[:], in_=res.ap()).then_inc(s_out, 16)

    nc.compile()
    return nc


def _get_nc():
    if "nc" not in _NC_CACHE:
        _NC_CACHE["nc"] = _build()
    return _NC_CACHE["nc"]


def _shard(output, targets):
    xs = np.ascontiguousarray(
        output.reshape(NCORES, NL, C).astype(np.float32, copy=False)
    )
    flat = np.arange(NL, dtype=np.int32) * C + targets.reshape(NCORES, NL).astype(
        np.int32
    )
    return xs, np.ascontiguousarray(flat.reshape(NCORES, P, F))


def _run(output, targets, **kwargs):
    xs, idx = _shard(output, targets)
    in_maps = [{"x": xs[c], "idx": idx[c]} for c in range(NCORES)]
    return run_bass_kernel_spmd(
        _get_nc(), in_maps, core_ids=list(range(NCORES)), **kwargs
    )


def kernel(output, targets):
    res = _run(output, targets)
    total = sum(float(r["partial"][0, 0]) for r in res.results)
    return np.array(np.float32(1.0) - np.float32(total / N), dtype=np.float32)
